# revision 4
# baseline (speedup 1.0000x reference)
"""AdptWeightBCEDiceLoss Trainium2 kernel v7.

Full inputs y_pred/y_target [32,1,512,512] f32 -> scalar f32 loss.
8 cores x 4 images data parallel.

v3 changes vs v2: single F tile (one tail Ln; 2 table loads, no scheduler
hoisting between sigmoids), all elementwise products on DVE stock
tensor_tensor (GpSimd tensor_tensor contends with DVE for the shared SBUF
port and slows both ~2.5x), pred*t5 moved to PE trace chains, pad memsets
on GpSimd, colsum packed into one [4,512] psum bank via indicator
stationary columns.

Per-core pipeline (images i=0..3, halves h=0,1), t5 = 5t bf16:
  SP  : input DMA image-ordered on sync HWDGE (t0, p0, consts, t1, p1, ...)
  GpS : pad memsets only
  DVE : BOXSUM scan -> sc; W_ABS custom: w = |D - t5| + 1, accum -> A;
        stock TT: Gw = w * t5 (2x bf16)
  PE  : h-pool band matmuls (10/img); pt-trace chains (pred x t5, global);
        paired FD-256 cex chains (F^T [w|Gw] diag -> sv, sx5);
        colsum(Gw) -> per-image su5 rows [4,512]
  ACT : sigmoid per image -> F[:, i, :]; ONE Ln over F (accum -> Sigma lnF);
        Copy evacs
Host combines in float64.
"""

import numpy as np

import concourse.bacc as bacc
import concourse.bass as bass
import concourse.tile as tile
from concourse import mybir
from concourse.bass_utils import run_bass_kernel_spmd

F32 = mybir.dt.float32
BF16 = mybir.dt.bfloat16

H = W = 512
RB = 4
KPOOL = 31
PADL = 32
SROW = 560
NPIX = H * W
N_CORES = 8
IMG_PER_CORE = 4
SMOOTH = 1e-8
QSCALE = 1.0 / (KPOOL * KPOOL)


def _register_op(name, spec):
    import concourse.dve_ops as DO
    from concourse.dve_spec import lower, _has_src1
    from concourse.dve_uop import DveOpSpec

    for op in DO.OPS:
        if op.name == name:
            return op
    probe = DO.DveOp(name, spec, subdim=False, uops_sha={})
    DO.OPS.append(probe)
    DO.CUSTOM_DVE_SPECS[name] = spec
    DO._SUB_OPCODE_FOR_NAME[name] = DO._CUSTOM_DVE_ROW_BASE + len(DO.OPS) - 1
    shas = {}
    for ver in ("v3", "v4"):
        r = DveOpSpec(
            name=name,
            opcode=DO.get_dve_sub_opcode(name),
            uops=lower(spec, ver=ver),
            rd1_en=_has_src1(spec),
        )
        shas[ver] = r.sha(ver)
    final = DO.DveOp(name, spec, subdim=False, uops_sha=shas)
    DO.OPS[-1] = final
    return final


def register_w_op():
    """W_ABS_ANT: out = |in0 - in1| + s1, accum = sum(out)."""
    from concourse.dve_spec import Spec, Src0, Src1, C1, Zero, maxx
    from operator import add as _add

    def ref(in0, in1, s0, s1, imm2):
        b = (np.abs(in0.astype(np.float32) - in1.astype(np.float32)) + s1).astype(
            np.float32
        )
        return b, b.reshape(b.shape[0], -1).sum(axis=-1, keepdims=True)

    return _register_op(
        "W_ABS_ANT",
        Spec(body=maxx(Src0 - Src1, Src1 - Src0) + C1,
             accum=_add, accum_init=Zero, reference=ref),
    )


def register_boxsum_op():
    from concourse.dve_spec import Spec, Src0, Src1, AluOp, scan

    def ref(in0, in1, s0, s1, imm2):
        return np.cumsum(in0.astype(np.float32) - in1.astype(np.float32),
                         axis=-1).astype(np.float32)

    return _register_op(
        "BOXSUM_SCAN_ANT",
        Spec(body=scan(AluOp.ADD, Src0 - Src1), reference=ref),
    )


def band_matrix_blocks() -> np.ndarray:
    import ml_dtypes

    pi = np.arange(128)[:, None]
    po = np.arange(128)[None, :]
    b0 = (np.abs(pi - po) <= 15).astype(np.float32)
    b1 = (pi - po >= 113).astype(np.float32)
    b2 = (po - pi >= 113).astype(np.float32)
    return np.ascontiguousarray(
        np.concatenate([b0, b1, b2], axis=1) * QSCALE
    ).astype(ml_dtypes.bfloat16)


def colsum_stationary() -> np.ndarray:
    """[128, 16] bf16: col 4i+j = 1 iff j == i (image indicator blocks)."""
    import ml_dtypes

    e = np.zeros((128, 16), dtype=np.float32)
    for i in range(4):
        e[:, 4 * i + i] = 1.0
    return e.astype(ml_dtypes.bfloat16)


def build_nc(n_img: int = IMG_PER_CORE) -> bacc.Bacc:
    register_w_op()
    register_boxsum_op()
    nc = bacc.Bacc("TRN2", target_bir_lowering=False, debug=False)
    pred_d = nc.dram_tensor("pb", [n_img, H, W], BF16, kind="ExternalInput")
    targ_d = nc.dram_tensor("tb5", [n_img, H, W], BF16, kind="ExternalInput")
    bb_d = nc.dram_tensor("bband", [128, 3 * 128], BF16, kind="ExternalInput")
    e4_d = nc.dram_tensor("ecols", [128, 16], BF16, kind="ExternalInput")
    acc_d = nc.dram_tensor("acc", [128, 16], F32, kind="ExternalOutput")
    cdg_d = nc.dram_tensor("cdg", [128, 1024], BF16, kind="ExternalOutput")
    ptc_d = nc.dram_tensor("ptc", [128, 256], BF16, kind="ExternalOutput")
    cs_d = nc.dram_tensor("csv", [4, 512], F32, kind="ExternalOutput")

    with tile.TileContext(nc) as tc:
        _body(tc, pred_d, targ_d, bb_d, e4_d, acc_d, cdg_d, ptc_d, cs_d, n_img)
    nc.compile()
    return nc


def _body(tc, pred_d, targ_d, bb_d, e4_d, acc_d, cdg_d, ptc_d, cs_d, n_img):
    W_OP = register_w_op()
    BOXSUM = register_boxsum_op()

    nc = tc.nc
    ACTF = mybir.ActivationFunctionType
    MULT = mybir.AluOpType.mult

    with (
        tc.tile_pool(name="const", bufs=1) as constp,
        tc.tile_pool(name="sc", bufs=1) as scp,
        tc.tile_pool(name="dps", bufs=1, space=bass.MemorySpace.PSUM) as dpsp,
        tc.tile_pool(name="chn", bufs=1, space=bass.MemorySpace.PSUM) as chnp,
    ):
        bb = constp.tile([128, 3 * 128], BF16)
        e4 = constp.tile([128, 16], BF16)
        acc = constp.tile([128, 16], F32)
        s1bufs = [constp.tile([128, RB, SROW], BF16, tag=f"s1_{k}", name=f"s1_{k}")
                  for k in range(4)]
        Pf = constp.tile([128, 4, 2048], BF16)
        Fb = constp.tile([128, 4, 2048], BF16)
        wgw_t = [constp.tile([128, RB, 2, 512], BF16, tag=f"wgw_{k}",
                             name=f"wgw_{k}") for k in range(n_img)]
        cdg = constp.tile([128, 1024], BF16)
        ptcv = constp.tile([128, 256], BF16)
        csv = constp.tile([4, 512], F32)

        # ---- input DMA, image-ordered, all on sync HWDGE ----------------
        def dma_t(i):
            nc.sync.dma_start(
                s1bufs[i][:, :, PADL:544],
                targ_d.ap()[i].rearrange("(rb p) w -> p rb w", p=128),
            )

        def dma_p(i):
            nc.sync.dma_start(
                Pf[:, i, :].rearrange("p (rb w) -> p rb w", w=W),
                pred_d.ap()[i].rearrange("(rb p) w -> p rb w", p=128),
            )

        dma_t(0)
        dma_t(1)
        nc.sync.dma_start(bb[:], bb_d.ap())
        nc.sync.dma_start(e4[:], e4_d.ap())
        dma_p(0)
        dma_t(2)
        dma_p(1)
        dma_t(3)
        dma_p(2)
        dma_p(3)

        # ---- priming: pads on GpSimd; DVE uop table; sigmoid table ------
        for k in range(4):
            nc.gpsimd.memset(s1bufs[k][:, :, 0:PADL], 0.0)
            nc.gpsimd.memset(s1bufs[k][:, :, 544:SROW], 0.0)
        zb = constp.tile([128, 1], F32)
        nc.vector.memset(zb[:], 0.0)
        pr0 = constp.tile([128, 1], F32)
        pr1 = constp.tile([128, 1], F32)
        nc.vector._custom_dve(W_OP, out=pr0[:], in0=zb[:], in1=zb[:],
                              s0=0.0, s1=1.0, accum_out=pr1[:])
        pra = constp.tile([128, 1], F32)
        nc.scalar.activation(pra[:], zb[:], ACTF.Sigmoid)

        # ---- psum tiles -------------------------------------------------
        cex = chnp.tile([128, 4, 2, 128], F32, tag="cex", name="cex")
        ptch = chnp.tile([128, 2, 128], F32, tag="ptch", name="ptch")
        cs4 = chnp.tile([4, 512], F32, tag="cs4", name="cs4")

        sc_t, dps_t = {}, {}

        def emit_scan(i):
            s1 = s1bufs[i]
            sc = scp.tile([128, RB, SROW], BF16, tag=f"sc_{i % 2}", name=f"sc_{i}")
            sc_t[i] = sc
            flat_in = s1[:].rearrange("p rb w -> p (rb w)")
            flat_out = sc[:].rearrange("p rb w -> p (rb w)")
            total = RB * SROW - (PADL + 1)
            nc.vector._custom_dve(
                BOXSUM, out=flat_out[:, 0:total],
                in0=flat_in[:, PADL:PADL + total],
                in1=flat_in[:, 1:1 + total],
            )

        def emit_hpool(i):
            sc = sc_t[i]
            dA = dpsp.tile([128, 2, 512], F32, tag="dps_h0", name=f"dA_{i}")
            dB = dpsp.tile([128, 2, 512], F32, tag="dps_h1", name=f"dB_{i}")
            dps_t[(i, 0)] = dA
            dps_t[(i, 1)] = dB
            out = lambda r: (dA if r < 2 else dB)[:, r % 2, :]
            for r in range(4):
                nc.tensor.matmul(out(r), bb[:, 0:128], sc[:, r, 15:527],
                                 start=True, stop=False)
            for r in range(1, 4):
                nc.tensor.matmul(out(r), bb[:, 128:256], sc[:, r - 1, 15:527],
                                 start=False, stop=(r == 3))
            for r in range(0, 3):
                nc.tensor.matmul(out(r), bb[:, 256:384], sc[:, r + 1, 15:527],
                                 start=False, stop=True)

        def emit_w(i, h):
            s1 = s1bufs[i]
            nc.vector._custom_dve(
                W_OP, out=wgw_t[i][:, 2 * h:2 * h + 2, 0, :],
                in0=dps_t[(i, h)][:],
                in1=s1[:, 2 * h:2 * h + 2, PADL:544],
                s0=0.0, s1=1.0,
                accum_out=acc[:, 2 * i + h:2 * i + h + 1],
            )

        def emit_sigmoid(i):
            nc.scalar.activation(Fb[:, i, :], Pf[:, i, :],
                                 ACTF.Sigmoid, scale=-1.0)

        def emit_gw(i, h):
            wgw = wgw_t[i]
            nc.vector.tensor_tensor(wgw[:, 2 * h:2 * h + 2, 1, :],
                                    wgw[:, 2 * h:2 * h + 2, 0, :],
                                    s1bufs[i][:, 2 * h:2 * h + 2, PADL:544], MULT)

        def emit_pttrace(i):
            # global pred*t5 diag chains, 2-bank rotation like the trace psum
            s1 = s1bufs[i]
            for blk in range(16):
                rb, cb = divmod(blk, 4)
                gblk = 16 * i + blk
                nc.tensor.matmul(
                    ptch[:, blk % 2, :],
                    Pf[:, i, blk * 128:(blk + 1) * 128],
                    s1[:, rb, PADL + cb * 128:PADL + (cb + 1) * 128],
                    start=(gblk < 2), stop=(gblk >= 16 * n_img - 2),
                )

        def emit_traces(i, h):
            wgw = wgw_t[i]
            for blk in range(8 * h, 8 * h + 8):
                rb, cb = divmod(blk, 4)
                Fblk = Fb[:, i, blk * 128:(blk + 1) * 128]
                nc.tensor.matmul(
                    cex[:, i, :, :], Fblk,
                    wgw[:, rb, :, cb * 128:(cb + 1) * 128],
                    start=(blk == 0), stop=(blk == 15),
                )

        def emit_colsum(i):
            gw = wgw_t[i]
            for rb in range(4):
                nc.tensor.matmul(
                    cs4[:], e4[:, 4 * i:4 * i + 4], gw[:, rb, 1, :],
                    start=(i == 0 and rb == 0), stop=(i == 3 and rb == 3),
                )

        # ---- software-pipelined emission --------------------------------
        emit_scan(0)
        emit_sigmoid(0)
        for i in range(n_img):
            if i + 1 < n_img:
                emit_scan(i + 1)
                emit_sigmoid(i + 1)
            emit_hpool(i)
            emit_pttrace(i)
            emit_w(i, 0)
            emit_gw(i, 0)
            emit_w(i, 1)
            emit_gw(i, 1)
            if i > 0:
                emit_traces(i - 1, 0)
                emit_traces(i - 1, 1)
                emit_colsum(i - 1)
        emit_traces(n_img - 1, 0)
        emit_colsum(n_img - 1)
        emit_traces(n_img - 1, 1)

        # ---- tail -------------------------------------------------------
        nc.scalar.activation(ptcv[:],
                             ptch[:].rearrange("p a b -> p (a b)"), ACTF.Copy)
        lndump = constp.tile([128, 8192], BF16)
        nc.scalar.activation(
            lndump[:], Fb[:].rearrange("p a b -> p (a b)"),
            ACTF.Ln, accum_out=acc[:, 8:9],
        )
        cexf = cex[:].rearrange("p a s b -> p (a s b)")
        nc.scalar.activation(cdg[:, 0:768], cexf[:, 0:768], ACTF.Copy)
        nc.scalar.activation(csv[:], cs4[:], ACTF.Copy)
        nc.scalar.activation(cdg[:, 768:1024], cexf[:, 768:1024], ACTF.Copy)

        nc.sync.dma_start(ptc_d.ap(), ptcv[:])
        nc.sync.dma_start(acc_d.ap(), acc[:])
        nc.sync.dma_start(cs_d.ap(), csv[:])
        nc.sync.dma_start(cdg_d.ap(), cdg[:])


def combine(results, n_img_total):
    n_img = IMG_PER_CORE
    per_image = []
    sp_total = 0.0
    pt_total = 0.0
    for r in results:
        a = r["acc"].astype(np.float64)
        cdg = r["cdg"].astype(np.float64).reshape(128, 4, 2, 128)
        ptc = r["ptc"].astype(np.float64).reshape(128, 2, 128)
        cs = r["csv"].astype(np.float64)
        sp_total += -a[:, 8:9].sum()
        pt_total += np.trace(ptc[:, 0, :]) + np.trace(ptc[:, 1, :])
        for i in range(n_img):
            A = a[:, 2 * i].sum() + a[:, 2 * i + 1].sum()
            sv = np.trace(cdg[:, i, 0, :])
            sx5 = np.trace(cdg[:, i, 1, :])
            su5 = cs[i, :].sum()
            per_image.append((A, sv, su5, sx5))
    bce = (sp_total - pt_total / 5.0) / (n_img_total * NPIX)
    total = 0.0
    for A, sv, su5, sx5 in per_image:
        B = (su5 - sx5) / 5.0
        C = A - sv + su5 / 5.0
        w_iou = 1.0 - (B + 1.0 + SMOOTH) / (C - B + 1.0 + SMOOTH)
        w_bce = (A * bce + SMOOTH) / (A + SMOOTH)
        total += w_bce + w_iou
    return np.float32(total / n_img_total)


def make_inputs(y_pred: np.ndarray, y_target: np.ndarray):
    import ml_dtypes

    pred = np.ascontiguousarray(np.asarray(y_pred, dtype=np.float32).reshape(-1, H, W))
    targ = np.ascontiguousarray(np.asarray(y_target, dtype=np.float32).reshape(-1, H, W))
    pb = pred.astype(ml_dtypes.bfloat16)
    tb5 = (5.0 * targ).astype(ml_dtypes.bfloat16)
    bb = band_matrix_blocks()
    e4 = colsum_stationary()
    in_maps = [
        {
            "pb": np.ascontiguousarray(pb[c * IMG_PER_CORE:(c + 1) * IMG_PER_CORE]),
            "tb5": np.ascontiguousarray(tb5[c * IMG_PER_CORE:(c + 1) * IMG_PER_CORE]),
            "bband": bb,
            "ecols": e4,
        }
        for c in range(N_CORES)
    ]
    return in_maps, pred.shape[0]


def kernel(y_pred: np.ndarray, y_target: np.ndarray) -> np.ndarray:
    in_maps, n_total = make_inputs(y_pred, y_target)
    nc = build_nc(IMG_PER_CORE)
    res = run_bass_kernel_spmd(nc, in_maps, list(range(N_CORES)))
    return np.asarray(combine([res.results[c] for c in range(N_CORES)], n_total))
